# revision 38
# baseline (speedup 1.0000x reference)
"""Trainium2 Bass kernel for DEMA (double exponential moving average) decomposition.

reference semantics (per batch row b, channel c, over time t):
    s0 = x[0], b0 = x[1] - x[0]
    for t in 1..T-1:
        s_t = alpha*x_t + (1-alpha)*(s_{t-1} + b_{t-1})
        b_t = beta*(s_t - s_{t-1}) + (1-beta)*b_{t-1}
    ma = [s0, s1, ..., s_{T-1}];  res = x - ma;  returns (res, ma)

The recurrence is linear in x, so it is restructured into dense matmuls:
time is split into NB=8 blocks of L=96.  With z_t = (s_t, b_t) and
z_t = A z_{t-1} + c x_t (A, c functions of alpha/beta only), one fused
constant stationary per block computes outputs AND the carry state:

    [ma_block0 ; Z_1]     = [W0 ; U0]        @ x_block0          (98 x 96)
    [ma_blockk ; Z_{k+1}] = [[W, PQ],[U, T2]] @ [x_blockk ; Z_k]  (98 x 98)

All matrices are computed on the host in float64 from alpha/beta.  The
carry Z_k is copied into 2 extra SBUF partitions (96:98) under x block k,
so each block is ONE float32r TensorE matmul; partition bases 96 are
32-aligned so engine copies of the carry rows are legal.

Sharding: batch dim (128) split across 8 cores (16 rows each); the
recurrence runs only over time so no cross-core communication is needed.
"""

import numpy as np
from contextlib import ExitStack

import bass_rust as _bass_rust
import concourse.bass as bass
import concourse.tile as tile
import concourse.tile_sem_assignment as _tsa
from concourse import mybir
from concourse.bass_utils import run_bass_kernel_spmd
from concourse.vector_clock import VectorClock, ScopedClock
from concourse.tile_scheduler import N_PROCS


N_CORES = 8
B, T, C = 128, 768, 256
L = 96
NB = T // L            # 8 time blocks
BS = B // N_CORES      # 16 batch rows per core
NPAIR = BS // 2        # batch rows processed in pairs
F32 = mybir.dt.float32
F32R = mybir.dt.float32r

# packed weight tensor [98, _WCOLS]: [W0AUG.T | WAUG2.T | VT_0..VT_6]
_WC_W0AUGT = 0         # [96, 98] in rows 0:96
_WC_WAUG2T = L + 2     # [98, 98]
_WC_VT = 2 * (L + 2)   # [96, 14] per j, rows 0:96
_WCOLS = 2 * (L + 2) + 14 * (NB - 1)


def _dema_matrices(alpha: float, beta: float):
    """Block-decomposition coefficient matrices, in float64."""
    a, b = float(alpha), float(beta)
    A = np.array([[1.0 - a, 1.0 - a], [-a * b, b * (1.0 - a) + (1.0 - b)]])
    c = np.array([a, a * b])
    e_s = np.array([1.0, 0.0])

    Apow = [np.eye(2)]
    for _ in range(T + 1):
        Apow.append(A @ Apow[-1])

    # generic block k>=1: z_{kL+t'} = A^{t'+1} Z_k + sum_{i'<=t'} A^{t'-i'} c x[kL+i']
    W = np.zeros((L, L))
    for tp in range(L):
        for ip in range(tp + 1):
            W[tp, ip] = (Apow[tp - ip] @ c)[0]
    PQ = np.zeros((L, 2))
    for tp in range(L):
        PQ[tp, :] = e_s @ Apow[tp + 1]
    U = np.zeros((2, L))
    for ip in range(L):
        U[:, ip] = Apow[L - 1 - ip] @ c
    T2 = Apow[L]

    # block 0: z_0 = (x_0, x_1 - x_0), ma_0 = x_0
    W0 = np.zeros((L, L))
    W0[0, 0] = 1.0
    M0 = np.array([[1.0, 0.0], [-1.0, 1.0]])
    for t in range(1, L):
        zc = Apow[t] @ M0
        W0[t, 0] += (e_s @ zc)[0]
        W0[t, 1] += (e_s @ zc)[1]
        for i in range(1, t + 1):
            W0[t, i] += (Apow[t - i] @ c)[0]
    U0 = np.zeros((2, L))
    zc = Apow[L - 1] @ M0
    U0[:, 0] += zc[:, 0]
    U0[:, 1] += zc[:, 1]
    for i in range(1, L):
        U0[:, i] += Apow[L - 1 - i] @ c

    W0AUG = np.vstack([W0, U0])                    # [98, 96]
    WAUG2 = np.block([[W, PQ], [U, T2]])           # [98, 98]

    # V_j: Z_all = sum_j V_j @ x_blockj where Z_k = sum_{j<k} T2^{k-1-j} Uj x_j
    T2pow = [np.eye(2)]
    for _ in range(NB):
        T2pow.append(T2 @ T2pow[-1])
    V = np.zeros((NB - 1, 2 * (NB - 1), L))
    for j in range(NB - 1):
        Uj = U0 if j == 0 else U
        for k in range(j + 1, NB):
            V[j, 2 * (k - 1) : 2 * k, :] = T2pow[k - 1 - j] @ Uj
    return W0AUG, WAUG2, V


def _pack_weights(alpha: float, beta: float) -> np.ndarray:
    W0AUG, WAUG2, V = _dema_matrices(alpha, beta)
    wts = np.zeros((L + 2, _WCOLS), dtype=np.float32)
    wts[0:L, _WC_W0AUGT : _WC_W0AUGT + L + 2] = W0AUG.T
    wts[:, _WC_WAUG2T : _WC_WAUG2T + L + 2] = WAUG2.T
    for j in range(NB - 1):
        wts[0:L, _WC_VT + 14 * j : _WC_VT + 14 * (j + 1)] = V[j].T
    return wts


def _wait_limit(inst) -> int:
    # walrus in this container rejects >1 sync wait on several instruction
    # formats (S3_LW, DMA DIRECT2D, CTRL); keep a single wait everywhere
    return 1


class _SplitDrainTC(tile.TileContext):
    """This walrus build rejects more than a couple of sync waits per
    instruction.  After scheduling + the stock kernel-tail drain, walk every
    block and move excess waits onto injected same-engine nops placed
    immediately before the over-limit instruction (waits execute on the
    engine sequencer before dispatch, so this is semantics-preserving)."""

    def _drain_and_barrier(self, tick_clock, wait_clock):
        super()._drain_and_barrier(tick_clock, wait_clock)
        self._split_excess_waits()

    def _split_excess_waits(self):
        nc = self.nc
        cur_list = nc.cur_bb.bb.instructions if nc.cur_bb is not None else None
        for fn in nc.m.functions:
            for blk in fn.blocks:
                insts = blk.instructions
                i = 0
                while i < len(insts):
                    inst = insts[i]
                    si = getattr(inst, "sync_info", None)
                    waits = list(si.on_wait) if si is not None else []
                    limit = _wait_limit(inst)
                    if len(waits) <= limit:
                        i += 1
                        continue
                    keep = waits[:limit]
                    excess = waits[limit:]
                    nops = []
                    for j in range(0, len(excess)):
                        nop = nc.engines[inst.engine].nop(nofuse=True).ins
                        # engine.nop() appended to the current bb; relocate it
                        if cur_list is not None and cur_list and cur_list[-1] is nop:
                            cur_list.pop()
                        nop.sync_info = _bass_rust.SyncInfo(
                            on_wait=excess[j : j + 1], on_update=[]
                        )
                        nops.append(nop)
                    si.on_wait = keep
                    insts[i:i] = nops
                    i += len(nops) + 1


def _pair_ap(dram, p: int):
    """contiguous [96 (partition), k*b'*c = 4096] view of tiled dram[p]."""
    return bass.AP(
        tensor=dram,
        offset=p * L * NB * 2 * C,
        ap=[[NB * 2 * C, L], [1, NB * 2 * C]],
    )


def _half_ap(dram, p: int, h: int):
    """[96 (partition), 4 blocks * b' * c = 2048] half-pair view of dram[p]."""
    return bass.AP(
        tensor=dram,
        offset=p * L * NB * 2 * C + h * (NB // 2) * 2 * C,
        ap=[[NB * 2 * C, L], [1, (NB // 2) * 2 * C]],
    )


def _build_nc() -> bass.Bass:
    nc = bass.Bass(trn_type="TRN2", target_bir_lowering=False, debug=False,
                   num_devices=N_CORES)
    # DRAM tensors use the SBUF tile layout [pair, t'=96, k, b', c] so every
    # DMA moves 16KB-contiguous runs per partition; the host does the
    # (cheap) permutation to/from [b, t, c] during shard/unshard.
    x_d = nc.dram_tensor("x", (NPAIR, L, NB, 2, C), F32R, kind="ExternalInput")
    w_d = nc.dram_tensor("wts", (L + 2, _WCOLS), F32R, kind="ExternalInput")
    res_d = nc.dram_tensor("res", (NPAIR, L, NB, 2, C), F32, kind="ExternalOutput")
    ma_d = nc.dram_tensor("ma", (NPAIR, L, NB, 2, C), F32, kind="ExternalOutput")

    with _SplitDrainTC(nc) as tc, ExitStack() as ctx:
        const = ctx.enter_context(tc.tile_pool(name="const", bufs=1))
        xbp = ctx.enter_context(tc.tile_pool(name="xb", bufs=6))
        masp = ctx.enter_context(tc.tile_pool(name="mas", bufs=4))
        resp = ctx.enter_context(tc.tile_pool(name="resb", bufs=4))
        zsp = ctx.enter_context(tc.tile_pool(name="zs", bufs=2))
        zps = ctx.enter_context(tc.tile_pool(name="zpsum", bufs=2, space="PSUM"))
        mps = ctx.enter_context(tc.tile_pool(name="mpsum", bufs=6, space="PSUM"))

        wts = const.tile([L + 2, _WCOLS], F32R)
        nc.sync.dma_start(wts[:], w_d.ap())
        w0augt = wts[0:L, _WC_W0AUGT : _WC_W0AUGT + L + 2]
        waug2t = wts[:, _WC_WAUG2T : _WC_WAUG2T + L + 2]
        vts = [wts[0:L, _WC_VT + 14 * j : _WC_VT + 14 * (j + 1)]
               for j in range(NB - 1)]

        # No serial carry chain: all block-entry states of a pair come from 7
        # V-matmuls, landed into the x tiles by tiny SBUF->SBUF DMAs (DMA is
        # exempt from the 32-aligned partition-base rule).  Emission is offset
        # by one pair (V-work of pair p before ma-work of pair p-1) so the PE
        # never waits on the carry placement.
        xbs, mass = {}, {}

        HF = NB // 2  # half-pair granularity (4 blocks) for DMA smoothing

        def emit_carry(p):
            xb = xbp.tile([L + 2, NB, 2, C], F32R, tag="xb", name=f"xb{p}")
            nc.sync.dma_start(
                bass.AP(tensor=xb.tensor, offset=xb.offset,
                        ap=[[xb.ap[0][0], L], [1, NB * 2 * C]]),
                _pair_ap(x_d, p),
            )
            xbs[p] = xb
            zb = zps.tile([2 * (NB - 1), 2, C], F32, tag="zb", name=f"zb{p}")
            for j in range(NB - 1):
                nc.tensor.matmul(zb[:], vts[j], xb[0:L, j],
                                 start=(j == 0), stop=(j == NB - 2))
            zs = zsp.tile([2 * (NB - 1), 2, C], F32R, tag="zs", name=f"zs{p}")
            nc.vector.tensor_copy(zs[:], zb[:])
            # SWDGE ring: keeps carry placement off the SyncE load ring (no
            # head-of-line blocking of the next pair's input DMAs).  One DMA
            # per state row: partitions {s, s+2, .., s+12} of zs fan out to
            # free-dim block slots k=1..7 of xb row 96+s.
            for srow in range(2):
                src = bass.AP(tensor=zs.tensor,
                              offset=zs.offset + srow * zs.ap[0][0],
                              ap=[[2 * zs.ap[0][0], NB - 1], [1, 2 * C]])
                dst = bass.AP(tensor=xb.tensor,
                              offset=xb.offset + (L + srow) * xb.ap[0][0]
                              + 2 * C,
                              ap=[[xb.ap[0][0], 1], [1, (NB - 1) * 2 * C]])
                nc.gpsimd.dma_start(dst, src)

        def emit_ma_half(p, h):
            xb, mas = xbs[p], mass[p]
            for k in range(h * HF, (h + 1) * HF):
                mapk = mps.tile([L, 2, C], F32, tag="mapk")
                if k == 0:
                    nc.tensor.matmul(mapk[:], w0augt[:, 0:L], xb[0:L, 0],
                                     start=True, stop=True)
                else:
                    nc.tensor.matmul(mapk[:], waug2t[:, 0:L], xb[:, k],
                                     start=True, stop=True)
                nc.scalar.copy(mas[:, k], mapk[:])
            ks = slice(h * HF, (h + 1) * HF)
            resb = resp.tile([L, HF, 2, C], F32, tag="resb")
            nc.vector.tensor_sub(resb[:], xb[0:L, ks].bitcast(F32), mas[:, ks])
            # outputs ride the ScalarE HWDGE ring so stores never queue ahead
            # of the next pair's loads on the SyncE ring
            nc.scalar.dma_start(_half_ap(ma_d, p, h), mas[:, ks])
            nc.scalar.dma_start(_half_ap(res_d, p, h), resb[:])

        for p in range(NPAIR + 1):
            if p < NPAIR:
                emit_carry(p)
            if p >= 1:
                mass[p - 1] = masp.tile([L, NB, 2, C], F32, tag="mas",
                                        name=f"mas{p - 1}")
                emit_ma_half(p - 1, 0)
                emit_ma_half(p - 1, 1)

    return nc


_NC_CACHE: bass.Bass | None = None


def _get_nc() -> bass.Bass:
    global _NC_CACHE
    if _NC_CACHE is None:
        _NC_CACHE = _build_nc()
    return _NC_CACHE


def _tile_layout(x_shard: np.ndarray) -> np.ndarray:
    """[BS, T, C] -> [NPAIR, L, NB, 2, C] tile-contiguous layout."""
    v = x_shard.reshape(NPAIR, 2, NB, L, C)
    return np.ascontiguousarray(v.transpose(0, 3, 2, 1, 4))


def _untile_layout(t: np.ndarray) -> np.ndarray:
    """[NPAIR, L, NB, 2, C] -> [BS, T, C]."""
    return t.transpose(0, 3, 2, 1, 4).reshape(BS, T, C)


def kernel(x: np.ndarray, alpha, beta):
    x = np.asarray(x, dtype=np.float32)
    assert x.shape == (B, T, C), x.shape
    wts = _pack_weights(float(alpha), float(beta))

    nc = _get_nc()
    in_maps = [
        {"x": _tile_layout(x[i * BS : (i + 1) * BS]), "wts": wts}
        for i in range(N_CORES)
    ]
    out = run_bass_kernel_spmd(nc, in_maps, core_ids=list(range(N_CORES)))
    res = np.concatenate(
        [_untile_layout(out.results[i]["res"]) for i in range(N_CORES)], axis=0
    )
    ma = np.concatenate(
        [_untile_layout(out.results[i]["ma"]) for i in range(N_CORES)], axis=0
    )
    return res, ma


# revision 40
# speedup vs baseline: 1.0678x; 1.0678x over previous
"""Trainium2 Bass kernel for DEMA (double exponential moving average) decomposition.

reference semantics (per batch row b, channel c, over time t):
    s0 = x[0], b0 = x[1] - x[0]
    for t in 1..T-1:
        s_t = alpha*x_t + (1-alpha)*(s_{t-1} + b_{t-1})
        b_t = beta*(s_t - s_{t-1}) + (1-beta)*b_{t-1}
    ma = [s0, s1, ..., s_{T-1}];  res = x - ma;  returns (res, ma)

The recurrence is linear in x, so it is restructured into dense matmuls:
time is split into NB=8 blocks of L=96.  With z_t = (s_t, b_t) and
z_t = A z_{t-1} + c x_t (A, c functions of alpha/beta only), one fused
constant stationary per block computes outputs AND the carry state:

    [ma_block0 ; Z_1]     = [W0 ; U0]        @ x_block0          (98 x 96)
    [ma_blockk ; Z_{k+1}] = [[W, PQ],[U, T2]] @ [x_blockk ; Z_k]  (98 x 98)

All matrices are computed on the host in float64 from alpha/beta.  The
carry Z_k is copied into 2 extra SBUF partitions (96:98) under x block k,
so each block is ONE float32r TensorE matmul; partition bases 96 are
32-aligned so engine copies of the carry rows are legal.

Sharding: batch dim (128) split across 8 cores (16 rows each); the
recurrence runs only over time so no cross-core communication is needed.
"""

import numpy as np
from contextlib import ExitStack

import bass_rust as _bass_rust
import concourse.bass as bass
import concourse.tile as tile
import concourse.tile_sem_assignment as _tsa
from concourse import mybir
from concourse.bass_utils import run_bass_kernel_spmd
from concourse.vector_clock import VectorClock, ScopedClock
from concourse.tile_scheduler import N_PROCS


N_CORES = 8
B, T, C = 128, 768, 256
L = 96
NB = T // L            # 8 time blocks
BS = B // N_CORES      # 16 batch rows per core
NPAIR = BS // 2        # batch rows processed in pairs
F32 = mybir.dt.float32
F32R = mybir.dt.float32r

# packed weight tensor [98, _WCOLS]: [W0AUG.T | WAUG2.T | VT_0..VT_6]
_WC_W0AUGT = 0         # [96, 98] in rows 0:96
_WC_WAUG2T = L + 2     # [98, 98]
_WC_VT = 2 * (L + 2)   # [96, 14] per j, rows 0:96
_WCOLS = 2 * (L + 2) + 14 * (NB - 1)


def _dema_matrices(alpha: float, beta: float):
    """Block-decomposition coefficient matrices, in float64."""
    a, b = float(alpha), float(beta)
    A = np.array([[1.0 - a, 1.0 - a], [-a * b, b * (1.0 - a) + (1.0 - b)]])
    c = np.array([a, a * b])
    e_s = np.array([1.0, 0.0])

    Apow = [np.eye(2)]
    for _ in range(T + 1):
        Apow.append(A @ Apow[-1])

    # generic block k>=1: z_{kL+t'} = A^{t'+1} Z_k + sum_{i'<=t'} A^{t'-i'} c x[kL+i']
    W = np.zeros((L, L))
    for tp in range(L):
        for ip in range(tp + 1):
            W[tp, ip] = (Apow[tp - ip] @ c)[0]
    PQ = np.zeros((L, 2))
    for tp in range(L):
        PQ[tp, :] = e_s @ Apow[tp + 1]
    U = np.zeros((2, L))
    for ip in range(L):
        U[:, ip] = Apow[L - 1 - ip] @ c
    T2 = Apow[L]

    # block 0: z_0 = (x_0, x_1 - x_0), ma_0 = x_0
    W0 = np.zeros((L, L))
    W0[0, 0] = 1.0
    M0 = np.array([[1.0, 0.0], [-1.0, 1.0]])
    for t in range(1, L):
        zc = Apow[t] @ M0
        W0[t, 0] += (e_s @ zc)[0]
        W0[t, 1] += (e_s @ zc)[1]
        for i in range(1, t + 1):
            W0[t, i] += (Apow[t - i] @ c)[0]
    U0 = np.zeros((2, L))
    zc = Apow[L - 1] @ M0
    U0[:, 0] += zc[:, 0]
    U0[:, 1] += zc[:, 1]
    for i in range(1, L):
        U0[:, i] += Apow[L - 1 - i] @ c

    W0AUG = np.vstack([W0, U0])                    # [98, 96]
    WAUG2 = np.block([[W, PQ], [U, T2]])           # [98, 98]

    # V_j: Z_all = sum_j V_j @ x_blockj where Z_k = sum_{j<k} T2^{k-1-j} Uj x_j
    T2pow = [np.eye(2)]
    for _ in range(NB):
        T2pow.append(T2 @ T2pow[-1])
    V = np.zeros((NB - 1, 2 * (NB - 1), L))
    for j in range(NB - 1):
        Uj = U0 if j == 0 else U
        for k in range(j + 1, NB):
            V[j, 2 * (k - 1) : 2 * k, :] = T2pow[k - 1 - j] @ Uj
    return W0AUG, WAUG2, V


def _pack_weights(alpha: float, beta: float) -> np.ndarray:
    W0AUG, WAUG2, V = _dema_matrices(alpha, beta)
    wts = np.zeros((L + 2, _WCOLS), dtype=np.float32)
    wts[0:L, _WC_W0AUGT : _WC_W0AUGT + L + 2] = W0AUG.T
    wts[:, _WC_WAUG2T : _WC_WAUG2T + L + 2] = WAUG2.T
    for j in range(NB - 1):
        wts[0:L, _WC_VT + 14 * j : _WC_VT + 14 * (j + 1)] = V[j].T
    return wts


def _wait_limit(inst) -> int:
    # walrus in this container rejects >1 sync wait on several instruction
    # formats (S3_LW, DMA DIRECT2D, CTRL); keep a single wait everywhere
    return 1


class _SplitDrainTC(tile.TileContext):
    """This walrus build rejects more than a couple of sync waits per
    instruction.  After scheduling + the stock kernel-tail drain, walk every
    block and move excess waits onto injected same-engine nops placed
    immediately before the over-limit instruction (waits execute on the
    engine sequencer before dispatch, so this is semantics-preserving)."""

    def _drain_and_barrier(self, tick_clock, wait_clock):
        super()._drain_and_barrier(tick_clock, wait_clock)
        self._split_excess_waits()

    def _split_excess_waits(self):
        nc = self.nc
        cur_list = nc.cur_bb.bb.instructions if nc.cur_bb is not None else None
        for fn in nc.m.functions:
            for blk in fn.blocks:
                insts = blk.instructions
                i = 0
                while i < len(insts):
                    inst = insts[i]
                    si = getattr(inst, "sync_info", None)
                    waits = list(si.on_wait) if si is not None else []
                    limit = _wait_limit(inst)
                    if len(waits) <= limit:
                        i += 1
                        continue
                    keep = waits[:limit]
                    excess = waits[limit:]
                    nops = []
                    for j in range(0, len(excess)):
                        nop = nc.engines[inst.engine].nop(nofuse=True).ins
                        # engine.nop() appended to the current bb; relocate it
                        if cur_list is not None and cur_list and cur_list[-1] is nop:
                            cur_list.pop()
                        nop.sync_info = _bass_rust.SyncInfo(
                            on_wait=excess[j : j + 1], on_update=[]
                        )
                        nops.append(nop)
                    si.on_wait = keep
                    insts[i:i] = nops
                    i += len(nops) + 1


def _pair_ap(dram, p: int):
    """contiguous [96 (partition), k*b'*c = 4096] view of tiled dram[p]."""
    return bass.AP(
        tensor=dram,
        offset=p * L * NB * 2 * C,
        ap=[[NB * 2 * C, L], [1, NB * 2 * C]],
    )


def _half_ap(dram, p: int, h: int):
    """[96 (partition), 4 blocks * b' * c = 2048] half-pair view of dram[p]."""
    return bass.AP(
        tensor=dram,
        offset=p * L * NB * 2 * C + h * (NB // 2) * 2 * C,
        ap=[[NB * 2 * C, L], [1, (NB // 2) * 2 * C]],
    )


def _build_nc() -> bass.Bass:
    nc = bass.Bass(trn_type="TRN2", target_bir_lowering=False, debug=False,
                   num_devices=N_CORES)
    # DRAM tensors use the SBUF tile layout [pair, t'=96, k, b', c] so every
    # DMA moves 16KB-contiguous runs per partition; the host does the
    # (cheap) permutation to/from [b, t, c] during shard/unshard.
    x_d = nc.dram_tensor("x", (NPAIR, L, NB, 2, C), F32R, kind="ExternalInput")
    w_d = nc.dram_tensor("wts", (L + 2, _WCOLS), F32R, kind="ExternalInput")
    res_d = nc.dram_tensor("res", (NPAIR, L, NB, 2, C), F32, kind="ExternalOutput")
    ma_d = nc.dram_tensor("ma", (NPAIR, L, NB, 2, C), F32, kind="ExternalOutput")

    with _SplitDrainTC(nc) as tc, ExitStack() as ctx:
        const = ctx.enter_context(tc.tile_pool(name="const", bufs=1))
        xbp = ctx.enter_context(tc.tile_pool(name="xb", bufs=6))
        masp = ctx.enter_context(tc.tile_pool(name="mas", bufs=4))
        resp = ctx.enter_context(tc.tile_pool(name="resb", bufs=4))
        zsp = ctx.enter_context(tc.tile_pool(name="zs", bufs=2))
        zps = ctx.enter_context(tc.tile_pool(name="zpsum", bufs=2, space="PSUM"))
        mps = ctx.enter_context(tc.tile_pool(name="mpsum", bufs=6, space="PSUM"))

        wts = const.tile([L + 2, _WCOLS], F32R)
        nc.sync.dma_start(wts[:], w_d.ap())
        w0augt = wts[0:L, _WC_W0AUGT : _WC_W0AUGT + L + 2]
        waug2t = wts[:, _WC_WAUG2T : _WC_WAUG2T + L + 2]
        vts = [wts[0:L, _WC_VT + 14 * j : _WC_VT + 14 * (j + 1)]
               for j in range(NB - 1)]

        # No serial carry chain: all block-entry states of a pair come from 7
        # V-matmuls, landed into the x tiles by tiny SBUF->SBUF DMAs (DMA is
        # exempt from the 32-aligned partition-base rule).  Emission is offset
        # by one pair (V-work of pair p before ma-work of pair p-1) so the PE
        # never waits on the carry placement.
        xbs, mass = {}, {}

        HF = NB // 2  # half-pair granularity (4 blocks) for DMA smoothing

        def emit_carry(p):
            xb = xbp.tile([L + 2, NB, 2, C], F32R, tag="xb", name=f"xb{p}")
            # quarter-pair loads: DRAM reads pipeline better at small runs
            # (measured: 8KB ins 17GB/s vs 16KB ins 15.5GB/s per engine) and
            # V-matmul j can start after quarter j//2 lands
            QC = 2 * 2 * C
            for q in range(4):
                nc.sync.dma_start(
                    bass.AP(tensor=xb.tensor,
                            offset=xb.offset + q * QC,
                            ap=[[xb.ap[0][0], L], [1, QC]]),
                    bass.AP(tensor=x_d,
                            offset=p * L * NB * 2 * C + q * QC,
                            ap=[[NB * 2 * C, L], [1, QC]]),
                )
            xbs[p] = xb
            zb = zps.tile([2 * (NB - 1), 2, C], F32, tag="zb", name=f"zb{p}")
            for j in range(NB - 1):
                nc.tensor.matmul(zb[:], vts[j], xb[0:L, j],
                                 start=(j == 0), stop=(j == NB - 2))
            zs = zsp.tile([2 * (NB - 1), 2, C], F32R, tag="zs", name=f"zs{p}")
            nc.vector.tensor_copy(zs[:], zb[:])
            # SWDGE ring: keeps carry placement off the SyncE load ring (no
            # head-of-line blocking of the next pair's input DMAs).  One DMA
            # per state row: partitions {s, s+2, .., s+12} of zs fan out to
            # free-dim block slots k=1..7 of xb row 96+s.
            for srow in range(2):
                src = bass.AP(tensor=zs.tensor,
                              offset=zs.offset + srow * zs.ap[0][0],
                              ap=[[2 * zs.ap[0][0], NB - 1], [1, 2 * C]])
                dst = bass.AP(tensor=xb.tensor,
                              offset=xb.offset + (L + srow) * xb.ap[0][0]
                              + 2 * C,
                              ap=[[xb.ap[0][0], 1], [1, (NB - 1) * 2 * C]])
                nc.gpsimd.dma_start(dst, src)

        def emit_ma_half(p, h):
            xb, mas = xbs[p], mass[p]
            for k in range(h * HF, (h + 1) * HF):
                mapk = mps.tile([L, 2, C], F32, tag="mapk")
                if k == 0:
                    nc.tensor.matmul(mapk[:], w0augt[:, 0:L], xb[0:L, 0],
                                     start=True, stop=True)
                else:
                    nc.tensor.matmul(mapk[:], waug2t[:, 0:L], xb[:, k],
                                     start=True, stop=True)
                nc.scalar.copy(mas[:, k], mapk[:])
            ks = slice(h * HF, (h + 1) * HF)
            resb = resp.tile([L, HF, 2, C], F32, tag="resb")
            nc.vector.tensor_sub(resb[:], xb[0:L, ks].bitcast(F32), mas[:, ks])
            # outputs ride the ScalarE HWDGE ring so stores never queue ahead
            # of the next pair's loads on the SyncE ring
            nc.scalar.dma_start(_half_ap(ma_d, p, h), mas[:, ks])
            nc.scalar.dma_start(_half_ap(res_d, p, h), resb[:])

        for p in range(NPAIR + 1):
            if p < NPAIR:
                emit_carry(p)
            if p >= 1:
                mass[p - 1] = masp.tile([L, NB, 2, C], F32, tag="mas",
                                        name=f"mas{p - 1}")
                emit_ma_half(p - 1, 0)
                emit_ma_half(p - 1, 1)

    return nc


_NC_CACHE: bass.Bass | None = None


def _get_nc() -> bass.Bass:
    global _NC_CACHE
    if _NC_CACHE is None:
        _NC_CACHE = _build_nc()
    return _NC_CACHE


def _tile_layout(x_shard: np.ndarray) -> np.ndarray:
    """[BS, T, C] -> [NPAIR, L, NB, 2, C] tile-contiguous layout."""
    v = x_shard.reshape(NPAIR, 2, NB, L, C)
    return np.ascontiguousarray(v.transpose(0, 3, 2, 1, 4))


def _untile_layout(t: np.ndarray) -> np.ndarray:
    """[NPAIR, L, NB, 2, C] -> [BS, T, C]."""
    return t.transpose(0, 3, 2, 1, 4).reshape(BS, T, C)


def kernel(x: np.ndarray, alpha, beta):
    x = np.asarray(x, dtype=np.float32)
    assert x.shape == (B, T, C), x.shape
    wts = _pack_weights(float(alpha), float(beta))

    nc = _get_nc()
    in_maps = [
        {"x": _tile_layout(x[i * BS : (i + 1) * BS]), "wts": wts}
        for i in range(N_CORES)
    ]
    out = run_bass_kernel_spmd(nc, in_maps, core_ids=list(range(N_CORES)))
    res = np.concatenate(
        [_untile_layout(out.results[i]["res"]) for i in range(N_CORES)], axis=0
    )
    ma = np.concatenate(
        [_untile_layout(out.results[i]["ma"]) for i in range(N_CORES)], axis=0
    )
    return res, ma
